# revision 1
# baseline (speedup 1.0000x reference)
"""Causal self-attention (B=4, T=1024, C=1024, H=16) on 8 Trainium2 cores.

Sharding: hybrid batch x head-group. Core c owns batch c//2 and head-group
c%2 (8 heads = 4 pairs of 2). Each core computes its batch's qkv slice,
causal attention for its 8 heads, and a partial projection (contraction
over its 512 rows of w_proj). Host sums the 2 partials per batch + b_proj.

This halves activation DMA vs pure head sharding: per-core traffic is
x-slice 4MB in + weights 8MB in + partial 4MB out.

Layout strategy (everything stays transposed so no on-chip transposes of
activations are ever needed except V):
  - x is fed as xTb [C, T] per batch; qkv matmuls produce q^T/k^T/v^T.
  - S^T[key, q] = k^T.T @ q^T per 128-key block, 2 heads row-packed in the
    128x128 PE array (d=64 each).
  - causal diag masking via a bf16 {0,-30000} matmul accumulated into S^T.
  - P^T = exp(S^T/8) (no max subtraction: logits are O(1) by construction).
  - O'^T accumulated as [V_h | ones].T @ P^T, so row 64 is the softmax
    denominator (f32r matmuls must write PSUM partition 0: no col packing).
  - y^T = O'^T * (1/denominator) via PE outer-product broadcast + DVE mul.
  - partial = y^T.T @ w_proj_rows (K=512 over 4 pair blocks), streamed out.

Matmuls use float32r (e8m11, full PE rate): ~2e-4 relative error.
"""

import numpy as np
import concourse.bass as bass
import concourse.mybir as mybir
import concourse.tile as tile
from concourse.bass import ts
from concourse.bass_utils import run_bass_kernel_spmd

F32 = mybir.dt.float32
F32R = mybir.dt.float32r
BF16 = mybir.dt.bfloat16
AF = mybir.ActivationFunctionType

B, T, C, H = 4, 1024, 1024, 16
D = C // H            # 64
NCORES = 8
HPC = 8               # heads per core
NPAIR = 4             # head pairs per core
NEG = -30000.0

_nc_cache = {}


def _split_sync_waits(nc):
    """This walrus build accepts exactly one sem-wait per instruction; move
    overflow waits onto fresh same-engine NoOps inserted just before."""
    n = 0
    for fn in nc.m.functions:
        for blk in fn.blocks:
            new_insts = []
            for inst in blk.instructions:
                si = getattr(inst, "sync_info", None)
                waits = list(si.on_wait) if si is not None and si.on_wait else []
                if len(waits) > 1:
                    for w in waits[1:]:
                        n += 1
                        new_insts.append(mybir.InstNoOp(
                            name=f"waitfix-{n}-{id(inst) & 0xffff}",
                            sync_info=mybir.SyncInfo(on_wait=[w], on_update=[]),
                            bass_nofuse=True,
                            engine=inst.engine,
                        ))
                    si.on_wait = waits[:1]
                new_insts.append(inst)
            blk.instructions[:] = new_insts
    return n


def build_nc(reps=1):
    xbufs, pbufs = 1, 4

    nc = bass.Bass()
    xT_d = nc.dram_tensor("xTb", [C, T], F32R, kind="ExternalInput")
    wqkv_d = nc.dram_tensor("wqkv", [C, NPAIR * 3 * 128], F32R, kind="ExternalInput")
    battn_d = nc.dram_tensor("battn", [128, NPAIR * 3], F32, kind="ExternalInput")
    wproj_d = nc.dram_tensor("wproj", [NPAIR * 128, C], F32R, kind="ExternalInput")
    idf_d = nc.dram_tensor("idf", [128, 128], F32, kind="ExternalInput")
    idb_d = nc.dram_tensor("idb", [128, 128], BF16, kind="ExternalInput")
    mask_d = nc.dram_tensor("mask", [128, 128], BF16, kind="ExternalInput")
    onescol_d = nc.dram_tensor("onescol", [128, 1], F32, kind="ExternalInput")
    onesrow_d = nc.dram_tensor("onesrow", [1, 64], F32R, kind="ExternalInput")
    out_d = nc.dram_tensor("partial", [T, C], F32, kind="ExternalOutput")

    CT = C // 128   # 8 contraction tiles
    KB = T // 128   # 8 key blocks
    QC = T // 512   # 2 query chunks

    with tile.TileContext(nc) as tc:
        with tc.tile_pool(name="const", bufs=1) as cpool, \
             tc.tile_pool(name="wp", bufs=1) as wpool, \
             tc.tile_pool(name="xp", bufs=xbufs) as xpool, \
             tc.tile_pool(name="qk", bufs=2) as qkpool, \
             tc.tile_pool(name="pp", bufs=pbufs) as ppool, \
             tc.tile_pool(name="yp", bufs=4) as ypool, \
             tc.tile_pool(name="op", bufs=pbufs) as opool, \
             tc.tile_pool(name="ps", bufs=1, space="PSUM") as ps:

            wqkv = wpool.tile([128, CT, 3 * NPAIR, 128], F32R)
            wqkv_src = wqkv_d.ap().rearrange(
                "(ct p) (mt m) -> p ct mt m", p=128, m=128)
            wproj = wpool.tile([128, NPAIR, 2, 512], F32R)
            battn = cpool.tile([128, NPAIR * 3], F32)
            nc.gpsimd.dma_start(out=battn, in_=battn_d.ap())
            idf = cpool.tile([128, 128], F32)
            idb = cpool.tile([128, 128], BF16)
            mask = cpool.tile([128, 128], BF16)
            onescol = cpool.tile([128, 1], F32)
            onesrow = cpool.tile([1, 64], F32R)
            nc.gpsimd.dma_start(out=idf, in_=idf_d.ap())
            nc.gpsimd.dma_start(out=idb, in_=idb_d.ap())
            nc.gpsimd.dma_start(out=mask, in_=mask_d.ap())
            nc.gpsimd.dma_start(out=onescol, in_=onescol_d.ap())
            nc.gpsimd.dma_start(out=onesrow, in_=onesrow_d.ap())

            for rep in range(reps):
                xT = xpool.tile([128, CT, 1024], F32R, tag="xT")
                for ct in range(CT):
                    if rep == 0:
                        # pair 0's weight blocks first so its qkv can stream;
                        # weights on the gpsimd (SWDGE) queue, x on SP (HWDGE)
                        nc.gpsimd.dma_start(out=wqkv[:, ct, 0:3, :],
                                            in_=wqkv_src[:, ct, 0:3, :])
                    nc.sync.dma_start(
                        out=xT[:, ct, :],
                        in_=xT_d.ap()[ct * 128:(ct + 1) * 128, :])
                if rep == 0:
                    for ct in range(CT):
                        nc.sync.dma_start(out=wqkv[:, ct, 3:12, :],
                                          in_=wqkv_src[:, ct, 3:12, :])
                    nc.sync.dma_start(out=wproj, in_=wproj_d.ap().rearrange(
                        "(pr p) (oh n) -> p pr oh n", p=128, n=512))

                yTs = []
                for pr in range(NPAIR):
                    # ---- QKV projection for this pair ----
                    qT = qkpool.tile([128, 1024], F32R, tag="qT")
                    kT = qkpool.tile([128, 1024], F32R, tag="kT")
                    vT = qkpool.tile([128, 1024], F32, tag="vT")
                    dests = ((0, qT), (1, kT), (2, vT))
                    if pr == 0:
                        # first pair streams against the incoming xT chunks:
                        # ct-outer order with 6 live accumulators (borrowing
                        # the attention psum tags, which are idle here)
                        accs = {}
                        slot = [("s_ps", 3), ("s_ps", 3), ("s_ps", 3),
                                ("bc", 1), ("mm512", 2), ("mm512", 2)]
                        for i, (mt, dest) in enumerate(dests):
                            for th in range(2):
                                tag, nb = slot[i * 2 + th]
                                accs[(mt, th)] = ps.tile(
                                    [128, 512], F32, tag=tag, bufs=nb,
                                    name=f"qacc_{rep}_{mt}_{th}")
                        for ct in range(CT):
                            for mt, dest in dests:
                                for th in range(2):
                                    nc.tensor.matmul(
                                        accs[(mt, th)], wqkv[:, ct, mt, :],
                                        xT[:, ct, ts(th, 512)],
                                        start=(ct == 0), stop=(ct == CT - 1),
                                        skip_group_check=True)
                        for mt, dest in dests:
                            for th in range(2):
                                nc.scalar.activation(
                                    out=dest[:, ts(th, 512)], in_=accs[(mt, th)],
                                    func=AF.Identity,
                                    bias=battn[:, mt:mt + 1], scale=1.0)
                    else:
                        for mt, dest in dests:
                            for th in range(2):
                                acc = ps.tile([128, 512], F32, tag="mm512", bufs=2)
                                for ct in range(CT):
                                    nc.tensor.matmul(
                                        acc, wqkv[:, ct, pr * 3 + mt, :],
                                        xT[:, ct, ts(th, 512)],
                                        start=(ct == 0), stop=(ct == CT - 1),
                                        skip_group_check=True)
                                nc.scalar.activation(
                                    out=dest[:, ts(th, 512)], in_=acc,
                                    func=AF.Identity,
                                    bias=battn[:, pr * 3 + mt:pr * 3 + mt + 1],
                                    scale=1.0)

                    # ---- V2e[keys, kb, [V_A|1|V_B|1]] via PE transpose ----
                    V2e = qkpool.tile([128, KB, 130], F32R, tag="V2e")
                    nc.vector.tensor_copy(out=V2e[:, :, 64:130:65],
                                          in_=onescol.to_broadcast([128, KB, 2]))
                    for kb in range(KB):
                        tp = ps.tile([128, 128], F32, tag="mm512", bufs=2)
                        nc.tensor.transpose(tp, vT[:, ts(kb, 128)], idf)
                        nc.vector.tensor_copy(out=V2e[:, kb, 0:64], in_=tp[:, 0:64])
                        nc.vector.tensor_copy(out=V2e[:, kb, 65:129],
                                              in_=tp[:, 64:128])

                    # ---- causal attention, 2 heads row-packed ----
                    yT = ypool.tile([128, 1024], F32R, tag="yT",
                                    name=f"yT_{rep}_{pr}")
                    yTs.append(yT)
                    for qc in range(QC):
                        osA = ps.tile([65, 512], F32, tag="os", bufs=2)
                        osB = ps.tile([65, 512], F32, tag="os", bufs=2)
                        kb_max = 4 * (qc + 1)
                        for kb in range(kb_max):
                            qoff = max(0, kb * 128 - qc * 512)
                            first = kb == 0
                            last = kb == kb_max - 1
                            sA = ps.tile([128, 512], F32, tag="s_ps", bufs=3)
                            sB = ps.tile([128, 512], F32, tag="s_ps", bufs=3)
                            qs = slice(qc * 512 + qoff, (qc + 1) * 512)
                            nc.tensor.matmul(sA[:, qoff:512],
                                             kT[0:64, ts(kb, 128)],
                                             qT[0:64, qs], start=True, stop=False,
                                             tile_position=(0, 0),
                                             skip_group_check=True)
                            nc.tensor.matmul(sB[:, qoff:512],
                                             kT[64:128, ts(kb, 128)],
                                             qT[64:128, qs], start=True, stop=False,
                                             tile_position=(64, 0),
                                             skip_group_check=True)
                            if kb * 128 >= qc * 512:   # diagonal block
                                nc.tensor.matmul(sA[:, qoff:qoff + 128], idb, mask,
                                                 start=False, stop=True,
                                                 skip_group_check=True)
                                nc.tensor.matmul(sB[:, qoff:qoff + 128], idb, mask,
                                                 start=False, stop=True,
                                                 skip_group_check=True)
                            pA = ppool.tile([128, 512], F32R, tag="pT")
                            pB = ppool.tile([128, 512], F32R, tag="pT")
                            nc.scalar.activation(out=pA[:, qoff:512],
                                                 in_=sA[:, qoff:512],
                                                 func=AF.Exp, scale=0.125)
                            nc.scalar.activation(out=pB[:, qoff:512],
                                                 in_=sB[:, qoff:512],
                                                 func=AF.Exp, scale=0.125)
                            nc.tensor.matmul(osA[:, qoff:512], V2e[:, kb, 0:65],
                                             pA[:, qoff:512], start=first,
                                             stop=last, skip_group_check=True)
                            nc.tensor.matmul(osB[:, qoff:512], V2e[:, kb, 65:130],
                                             pB[:, qoff:512], start=first,
                                             stop=last, skip_group_check=True)
                        recA = ppool.tile([1, 512], F32R, tag="rec", bufs=4)
                        recB = ppool.tile([1, 512], F32R, tag="rec", bufs=4)
                        with nc.allow_low_precision(reason="f32r softmax denom"):
                            nc.vector.reciprocal(out=recA, in_=osA[64:65, :])
                            nc.vector.reciprocal(out=recB, in_=osB[64:65, :])
                        bcA = ps.tile([64, 512], F32, tag="bc", bufs=1)
                        bcB = ps.tile([64, 512], F32, tag="bc", bufs=1)
                        nc.tensor.matmul(bcA, onesrow, recA, start=True, stop=True,
                                         skip_group_check=True)
                        nc.tensor.matmul(bcB, onesrow, recB, start=True, stop=True,
                                         skip_group_check=True)
                        bcA_sb = ppool.tile([64, 512], F32, tag="bcsb", bufs=2)
                        bcB_sb = ppool.tile([64, 512], F32, tag="bcsb", bufs=2)
                        nc.vector.tensor_copy(out=bcA_sb, in_=bcA)
                        nc.vector.tensor_copy(out=bcB_sb, in_=bcB)
                        nc.vector.tensor_mul(yT[0:64, ts(qc, 512)],
                                             osA[0:64, :], bcA_sb)
                        nc.vector.tensor_mul(yT[64:128, ts(qc, 512)],
                                             osB[0:64, :], bcB_sb)

                        # once every pair finished a token chunk, project it
                        if pr == NPAIR - 1:
                            for tt in range(4 * qc, 4 * qc + 4):
                                for oh in range(2):
                                    pp = ps.tile([128, 512], F32, tag="mm512",
                                                 bufs=2, name=f"pp_{rep}_{tt}_{oh}")
                                    for pj in range(NPAIR):
                                        nc.tensor.matmul(
                                            pp, yTs[pj][:, ts(tt, 128)],
                                            wproj[:, pj, oh, :],
                                            start=(pj == 0), stop=(pj == NPAIR - 1),
                                            skip_group_check=True)
                                    ot = opool.tile([128, 512], F32, tag="ot",
                                                    name=f"ot_{rep}_{tt}_{oh}")
                                    if (tt + oh) % 2 == 0:
                                        nc.vector.tensor_copy(out=ot, in_=pp)
                                    else:
                                        nc.scalar.copy(out=ot, in_=pp)
                                    nc.gpsimd.dma_start(
                                        out=out_d.ap()[tt * 128:(tt + 1) * 128,
                                                       ts(oh, 512)],
                                        in_=ot)
    _split_sync_waits(nc)
    return nc


def make_in_maps(x, w_attn, b_attn, w_proj):
    xT = x.reshape(B * T, C).T                                 # [C, B*T] view
    idf = np.eye(128, dtype=np.float32)

    def to_bf16(a):
        import ml_dtypes
        return a.astype(ml_dtypes.bfloat16)
    idb = to_bf16(np.eye(128, dtype=np.float32))
    maskb = to_bf16(np.tril(np.full((128, 128), NEG, dtype=np.float32), -1))
    onescol = np.ones((128, 1), dtype=np.float32)
    onesrow = np.ones((1, 64), dtype=np.float32)

    xTb = [np.ascontiguousarray(xT[:, b * T:(b + 1) * T]) for b in range(B)]
    in_maps = []
    for c in range(NCORES):
        bi, hg = divmod(c, 2)
        blocks, bias_cols, wp = [], [], []
        for pr in range(NPAIR):
            h0 = (hg * 8 + pr * 2) * D
            blocks += [w_attn[:, h0:h0 + 128],
                       w_attn[:, C + h0:C + h0 + 128],
                       w_attn[:, 2 * C + h0:2 * C + h0 + 128]]
            bias_cols += [b_attn[h0:h0 + 128],
                          b_attn[C + h0:C + h0 + 128],
                          b_attn[2 * C + h0:2 * C + h0 + 128]]
            wp.append(w_proj[h0:h0 + 128, :])
        wqkv = np.ascontiguousarray(np.concatenate(blocks, axis=1))   # [C, 1536]
        battn = np.stack(bias_cols, axis=1).astype(np.float32)        # [128, 12]
        wprojc = np.ascontiguousarray(np.concatenate(wp, axis=0))     # [512, C]
        in_maps.append({
            "xTb": xTb[bi], "wqkv": wqkv, "battn": battn, "wproj": wprojc,
            "idf": idf, "idb": idb, "mask": maskb,
            "onescol": onescol, "onesrow": onesrow,
        })
    return in_maps


def kernel(x, w_attn, b_attn, w_proj, b_proj):
    x = np.asarray(x)
    w_attn = np.asarray(w_attn)
    b_attn = np.asarray(b_attn)
    w_proj = np.asarray(w_proj)
    b_proj = np.asarray(b_proj)

    if "nc" not in _nc_cache:
        _nc_cache["nc"] = build_nc()
    nc = _nc_cache["nc"]
    in_maps = make_in_maps(x, w_attn, b_attn, w_proj)

    res = run_bass_kernel_spmd(nc, in_maps, core_ids=list(range(NCORES)))
    out = np.empty((B, T, C), dtype=np.float32)
    for bi in range(B):
        out[bi] = res.results[2 * bi]["partial"]
        out[bi] += res.results[2 * bi + 1]["partial"]
        out[bi] += b_proj.astype(np.float32)
    return out

